# revision 11
# baseline (speedup 1.0000x reference)
"""Causal self-attention (B=4, T=2048, D=1024, H=16) on 8 trn2 NeuronCores.

Sharding: tensor-parallel over heads - 2 heads per core. Each core computes
qkv projections for its 2 heads (from replicated x), causal attention, and a
partial output projection (its 128 rows of w_proj). Host sums the 8 partial
[S, D] outputs.

v2 structure (bf16 into the PE, fp32 PSUM accum only where needed):
  - QKV: per 512-row sub-chunk, q/k accumulate in 2 fp32 psum banks, v in a
    third; interleaved into the attention stream as PE filler.
  - v blocks: DMA-transpose (XBAR, dst 16-col aligned) from vt staging into
    [v0 | 1 | pad | v1@80 | 1 | pad] blocks; AV lhsT = [80h : 80h+65].
  - scores: groups of 2 j-tiles; both heads + both jts in one [128,2048]
    bf16 psum tile (bank0=h0, bank1=h1 so the two K=64 row-tiled matmuls
    can run concurrently); ONE wide exp per group -> p bf16 in SBUF.
  - AV: deferred per-head bursts over the whole chunk (128-mode
    back-to-back), accumulating [o_h | den_h] in one fp32 bank.
  - epilogue per chunk: den -> [8,128] via reshape-DMA, cheap reciprocal,
    broadcast via K=8 matmuls, normalize oT in SBUF, project into bf16
    psum, one wide copy per m-tile, DMA out. The epilogue of chunk X is
    emitted as PE filler inside chunk X+1's (ScalarE-paced) score phase.
"""

import math

import numpy as np
import ml_dtypes

B, T, D, H = 4, 2048, 1024, 16
HD = D // H           # 64
S = B * T             # 8192
P = 128
KT = D // P           # 8 k-tiles over D
SC = S // 512         # 16 qkv sub-chunks of 512 rows
JT = T // P           # 16 j-tiles per batch
NCH = T // 512        # 4 i-chunks per batch
NT = S // P           # 64 m-tiles of 128
N_CORES = 8

BFNP = ml_dtypes.bfloat16

_CACHE = {}


def _build_nc():
    import concourse.tile as tile
    import concourse.mybir as mybir
    from concourse import bacc

    BF = mybir.dt.bfloat16
    F32 = mybir.dt.float32
    Exp = mybir.ActivationFunctionType.Exp
    SCALE = 1.0 / math.sqrt(HD)

    nc = bacc.Bacc("TRN2", num_devices=N_CORES)

    xt = nc.dram_tensor("xt", [SC, P, KT * 512], BF, kind="ExternalInput").ap()
    wq = nc.dram_tensor("wq", [D, P], BF, kind="ExternalInput").ap()
    wk = nc.dram_tensor("wk", [D, P], BF, kind="ExternalInput").ap()
    wv = nc.dram_tensor("wv", [D, P], BF, kind="ExternalInput").ap()
    wp = nc.dram_tensor("wp", [P, D], BF, kind="ExternalInput").ap()
    maskt = nc.dram_tensor("maskt", [P, P], BF, kind="ExternalInput").ap()
    ebc = nc.dram_tensor("ebc", [8, NCH * P], BF, kind="ExternalInput").ap()
    out_p = nc.dram_tensor("out_p", [S, D], BF, kind="ExternalOutput").ap()

    with tile.TileContext(nc) as tc:
        with tc.tile_pool(name="singles", bufs=1) as singles:
            qT_sb = singles.tile([P, S], BF)
            kT_sb = singles.tile([P, S], BF)
            # v blocks per m-tile: interleaved [v0 v1 v0 v1 ... | 1 1]
            v_sb = singles.tile([P, NT, 160], BF)
            wq_sb = singles.tile([P, KT, P], BF)
            wk_sb = singles.tile([P, KT, P], BF)
            wv_sb = singles.tile([P, KT, P], BF)
            wp_sb = singles.tile([P, D], BF)
            mask_sb = singles.tile([P, P], BF)
            e_sb = singles.tile([8, NCH * P], BF)

            nc.sync.dma_start(out=wq_sb, in_=wq.rearrange("(kt p) n -> p kt n", p=P))
            nc.sync.dma_start(out=wk_sb, in_=wk.rearrange("(kt p) n -> p kt n", p=P))
            nc.sync.dma_start(out=wv_sb, in_=wv.rearrange("(kt p) n -> p kt n", p=P))
            nc.sync.dma_start(out=wp_sb, in_=wp)
            nc.sync.dma_start(out=mask_sb, in_=maskt)
            nc.sync.dma_start(out=e_sb, in_=ebc)
            nc.vector.memset(v_sb[:, :, 64:65], 1.0)
            nc.vector.memset(v_sb[:, :, 144:145], 1.0)

            with (
                tc.tile_pool(name="xc_pool", bufs=2) as xpool,
                tc.tile_pool(name="vt_pool", bufs=2) as vtp,
                tc.tile_pool(name="p_pool", bufs=34) as ppool,
                tc.tile_pool(name="oT_pool", bufs=2) as otp,
                tc.tile_pool(name="d_pool", bufs=2) as dpool,
                tc.tile_pool(name="g_pool", bufs=6) as gpool,
                tc.tile_pool(name="ob_pool", bufs=3) as obp,
                # PSUM budget (8 banks): scores 2x2 + av 1 + qk/bc/pj 2 + v 1
                tc.tile_pool(name="ps_s", bufs=2, space="PSUM") as ps_s,
                tc.tile_pool(name="ps_av", bufs=1, space="PSUM") as ps_av,
                tc.tile_pool(name="ps_m", bufs=2, space="PSUM") as ps_m,
                tc.tile_pool(name="ps_v", bufs=1, space="PSUM") as ps_v,
            ):
                def qkv_subchunk(sc):
                    sl = slice(sc * 512, (sc + 1) * 512)
                    xc = xpool.tile([P, KT * 512], BF, name="xc")
                    nc.sync.dma_start(out=xc, in_=xt[sc])
                    q_ps = ps_m.tile([P, 512], F32, name="q_ps", tag="psm")
                    k_ps = ps_m.tile([P, 512], F32, name="k_ps", tag="psm")
                    for kt in range(KT):
                        xk = xc[:, kt * 512:(kt + 1) * 512]
                        nc.tensor.matmul(q_ps, lhsT=wq_sb[:, kt], rhs=xk,
                                         start=(kt == 0), stop=(kt == KT - 1))
                        nc.tensor.matmul(k_ps, lhsT=wk_sb[:, kt], rhs=xk,
                                         start=(kt == 0), stop=(kt == KT - 1))
                    nc.vector.tensor_copy(out=qT_sb[:, sl], in_=q_ps)
                    nc.vector.tensor_copy(out=kT_sb[:, sl], in_=k_ps)
                    v_ps = ps_v.tile([P, 512], F32, name="v_ps", tag="psv")
                    for kt in range(KT):
                        nc.tensor.matmul(v_ps, lhsT=wv_sb[:, kt],
                                         rhs=xc[:, kt * 512:(kt + 1) * 512],
                                         start=(kt == 0), stop=(kt == KT - 1))
                    vt = vtp.tile([P, 512], BF, name="vt")
                    nc.scalar.copy(out=vt, in_=v_ps)
                    # XBAR transpose into v blocks [v0 | 1 | v1 | 1]
                    for i in range(4):
                        mt = sc * 4 + i
                        for h in (0, 1):
                            nc.sync.dma_start(
                                out=v_sb[:, mt, 80 * h:80 * h + 64],
                                in_=vt[h * 64:(h + 1) * 64, i * P:(i + 1) * P],
                                transpose=True)

                def attn_chunk(b, c, fillers):
                    base = b * T
                    i0 = base + c * 512
                    njt = 4 * c + 4
                    p_tiles = []
                    # ---- phase A: scores (64-row-tiled pairs) + exp ----
                    for jt in range(njt):
                        diag = jt >= 4 * c
                        off = jt * P - c * 512 if diag else 0
                        s_t = ps_s.tile([P, 1024], F32, name="s_t")
                        for h in (0, 1):
                            lk = kT_sb[h * 64:(h + 1) * 64,
                                       base + jt * P: base + (jt + 1) * P]
                            rq = qT_sb[h * 64:(h + 1) * 64, i0 + off: i0 + 512]
                            nc.tensor.matmul(
                                s_t[:, 512 * h + off: 512 * (h + 1)],
                                lhsT=lk, rhs=rq, start=True, stop=True,
                                tile_position=(64 * h, 0))
                        p_t = ppool.tile([P, 1024], BF, name="p_t")
                        p_tiles.append(p_t)
                        if off < 172:
                            # single exp over both heads (junk gap unread)
                            nc.scalar.activation(out=p_t[:, off:1024],
                                                 in_=s_t[:, off:1024],
                                                 func=Exp, scale=SCALE)
                        else:
                            for h in (0, 1):
                                nc.scalar.activation(
                                    out=p_t[:, 512 * h + off: 512 * (h + 1)],
                                    in_=s_t[:, 512 * h + off: 512 * (h + 1)],
                                    func=Exp, scale=SCALE)
                        if diag:
                            for h in (0, 1):
                                lo = 512 * h + off
                                nc.gpsimd.tensor_mul(
                                    out=p_t[:, lo:lo + P],
                                    in0=p_t[:, lo:lo + P], in1=mask_sb)
                        if fillers and jt % 2 == 1:
                            fillers.pop(0)()
                    while fillers:
                        fillers.pop(0)()
                    # ---- phase B: per-head AV bursts + den/oT evacuation ----
                    dstage = dpool.tile([1, 1024], F32, name="dstage")
                    oT_cb = otp.tile([P, 512], BF, name="oT_cb")
                    for h in (0, 1):
                        av = ps_av.tile([P, 512], F32, name="av")
                        for jt in range(njt):
                            off = jt * P - c * 512 if jt >= 4 * c else 0
                            lv = v_sb[:, b * JT + jt, 80 * h:80 * h + 65]
                            nc.tensor.matmul(
                                av[0:65, off:512],
                                lhsT=lv,
                                rhs=p_tiles[jt][:, 512 * h + off: 512 * (h + 1)],
                                start=(jt == 0), stop=(jt == njt - 1))
                        nc.vector.tensor_copy(out=oT_cb[h * 64:(h + 1) * 64, :],
                                              in_=av[0:64])
                        nc.vector.tensor_copy(out=dstage[:, h * 512:(h + 1) * 512],
                                              in_=av[64:65])
                    # ---- reciprocal of denominators in [8, 128] layout ----
                    g_cb = gpool.tile([8, P], F32, name="g_cb")
                    for h in (0, 1):
                        nc.sync.dma_start(out=g_cb[h * 4:(h + 1) * 4, :],
                                          in_=dstage[0:1, h * 512:(h + 1) * 512])
                    r_cb = gpool.tile([8, P], F32, name="r_cb")
                    nc.vector.reciprocal(out=r_cb, in_=g_cb)
                    rb_cb = gpool.tile([8, P], BF, name="rb_cb")
                    nc.vector.tensor_copy(out=rb_cb, in_=r_cb)

                    def epilogue():
                        # broadcast recip over the 128 head-dims, normalize,
                        # project, ship out
                        bc = ps_m.tile([P, 512], F32, name="bc", tag="psm")
                        for it in range(4):
                            nc.tensor.matmul(bc[:, it * P:(it + 1) * P],
                                             lhsT=e_sb[:, it * P:(it + 1) * P],
                                             rhs=rb_cb, start=True, stop=True)
                        nc.vector.tensor_mul(out=oT_cb, in0=oT_cb, in1=bc)
                        for i in range(4):
                            mt = (b * T + c * 512) // P + i
                            ob = obp.tile([P, D], BF, name="ob")
                            for nch in range(2):
                                pj = ps_m.tile([P, 512], F32, name="pj",
                                               tag="psm")
                                nc.tensor.matmul(
                                    pj,
                                    lhsT=oT_cb[:, i * P:(i + 1) * P],
                                    rhs=wp_sb[:, nch * 512:(nch + 1) * 512],
                                    start=True, stop=True)
                                nc.vector.tensor_copy(
                                    out=ob[:, nch * 512:(nch + 1) * 512], in_=pj)
                            nc.sync.dma_start(out=out_p[mt * P:(mt + 1) * P, :],
                                              in_=ob)
                    return epilogue

                # batch 0 qkv up front; batch b+1 qkv + previous chunk's
                # epilogue run as PE filler inside each chunk's score phase
                for sc in range(4):
                    qkv_subchunk(sc)
                prev_epi = None
                for b in range(B):
                    for c in range(NCH):
                        fillers = []
                        sc = 4 * (b + 1) + c
                        if sc < SC:
                            fillers.append(lambda s=sc: qkv_subchunk(s))
                        if prev_epi is not None:
                            fillers.append(prev_epi)
                        prev_epi = attn_chunk(b, c, fillers)
                prev_epi()

    nc.compile()
    return nc


def _host_inputs(x, w_qkv, w_proj):
    x = np.asarray(x, dtype=np.float32)
    w_qkv = np.asarray(w_qkv, dtype=np.float32)
    w_proj = np.asarray(w_proj, dtype=np.float32)

    xT = np.ascontiguousarray(x.reshape(S, D).T).astype(BFNP)
    # [sc, p, kt*512+j] = xT[kt*128+p, sc*512+j]
    xt = np.ascontiguousarray(
        xT.reshape(KT, P, SC, 512).transpose(2, 1, 0, 3).reshape(SC, P, KT * 512))
    mask = np.triu(np.ones((P, P), np.float32)).astype(BFNP)  # [j, i]: 1 if j<=i
    # bc matmul: out[m, i] = sum_r E[r, m] rb[r, i]; want rb[h(m)*4 + it, i]
    ebc = np.zeros((8, NCH, P), np.float32)
    for it in range(NCH):
        ebc[it, it, 0:64] = 1.0
        ebc[4 + it, it, 64:128] = 1.0
    ebc = ebc.reshape(8, NCH * P).astype(BFNP)

    in_maps = []
    for core in range(N_CORES):
        cs = slice(core * P, (core + 1) * P)
        in_maps.append({
            "xt": xt,
            "wq": np.ascontiguousarray(w_qkv[:, core * P:(core + 1) * P]).astype(BFNP),
            "wk": np.ascontiguousarray(w_qkv[:, D + core * P: D + (core + 1) * P]).astype(BFNP),
            "wv": np.ascontiguousarray(w_qkv[:, 2 * D + core * P: 2 * D + (core + 1) * P]).astype(BFNP),
            "wp": np.ascontiguousarray(w_proj[cs, :]).astype(BFNP),
            "maskt": mask,
            "ebc": ebc,
        })
    return in_maps


def run_spmd(x, w_qkv, w_proj, trace=False):
    """Compile (cached) + run on 8 cores. Returns (out [B,T,D] fp32, results)."""
    from concourse import bass_utils

    if "nc" not in _CACHE:
        _CACHE["nc"] = _build_nc()
    nc = _CACHE["nc"]

    in_maps = _host_inputs(x, w_qkv, w_proj)
    res = bass_utils.run_bass_kernel_spmd(
        nc, in_maps, core_ids=list(range(N_CORES)), trace=trace)

    acc = np.zeros((S, D), np.float32)
    for r in res.results:
        acc += np.asarray(r["out_p"]).astype(np.float32)
    return acc.reshape(B, T, D), res


def kernel(x, w_qkv, w_proj):
    out, _ = run_spmd(x, w_qkv, w_proj, trace=False)
    return out
